# revision 16
# baseline (speedup 1.0000x reference)
"""Trainium2 Bass kernel for nn_BiDiBoundaryPredictor.

Math: logits = x @ W + b; a sequential per-timestep scan adds NEG=-10000 to the
boundary logit while a 4-state refractory counter (flag in {0..3}) is nonzero;
outputs are argmax over the masked logits and the log-softmax probability at
the argmax (temperature TAU=2).

Only the logit DIFFERENCE delta = x . (W[:,1]-W[:,0]) + (b1-b0) matters:
  pred_t      = (delta_t + NEG*mask_t) > 0
  gathered_t  = -ln(1 + exp(-|delta_t + NEG*mask_t| / TAU))
  mask_t      = (flag_t > 0), flag driven by bits d_t = delta_t > 0.

Per-core plan (8 cores, 2 sequences each; data-parallel over batch):
  Phase 1 (DMA-bound): stream x in 1 MiB tiles; DVE tensor_tensor_reduce
    computes y = sum(x*w)+bd per 128-timestep column -> y_all [128,128].
  Phase 2: one PE transpose -> ys [lane,128] where lane=(seq,chunk-of-128).
    The 4-state automaton is an associative map composition, computed with a
    Blelloch scan over map images (4 values/map): in-lane up-sweep (bootstrap
    4-step blocks via a first-one formula, then gather-combines), a cross-lane
    scan over the 128 chunk summaries, in-lane down-sweep, then closed-form
    per-position masks. Epilogue on ACT: Abs/Exp/Ln for gathered.
"""
import numpy as np
from contextlib import ExitStack

import concourse.bass as bass
import concourse.tile as tile
from concourse import bacc, mybir
from concourse._compat import with_exitstack
from concourse.masks import make_identity
from concourse.bass_utils import run_bass_kernel_spmd

F32 = mybir.dt.float32
Alu = mybir.AluOpType
Act = mybir.ActivationFunctionType

NEG = -10000.0
TAU = 2.0
N_CORES = 8
BS, L, D = 16, 8192, 512
SEQ_PER_CORE = BS // N_CORES          # 2
ROWS = SEQ_PER_CORE * L               # 16384 x-rows per core
CHUNK = 128                           # timesteps per lane
NLANE = ROWS // CHUNK                 # 128 lanes = (seq, chunk)
LANES_PER_SEQ = L // CHUNK            # 64
TILE_A = 4                            # 128-row chunks per DMA tile (1 MiB)
NBLK = CHUNK // 4                     # 32 four-step blocks per lane
PAD = 5                               # padded block stride for map storage


def _ev(t, n):
    """[P, n, 4] entry view of a PAD-strided map tile."""
    return t[:].rearrange("p (b e) -> p b e", e=PAD)[:, :n, 0:4]


def _ej(t, n, j):
    """[P, n, 1] entry-j view."""
    return t[:].rearrange("p (b e) -> p b e", e=PAD)[:, :n, j:j + 1]


@with_exitstack
def _program(ctx: ExitStack, tc: tile.TileContext,
             x_in, w_in, b_in, bnd_out, gth_out):
    nc = tc.nc

    xpool = ctx.enter_context(tc.tile_pool(name="x", bufs=3))
    ppool = ctx.enter_context(tc.tile_pool(name="prod", bufs=2))
    wpool = ctx.enter_context(tc.tile_pool(name="w", bufs=1))
    spool = ctx.enter_context(tc.tile_pool(name="scan", bufs=1))
    psum = ctx.enter_context(tc.tile_pool(name="ps", bufs=1, space="PSUM"))

    wt = wpool.tile([128, D], F32)
    nc.sync.dma_start(wt[:], w_in[:])
    bt = wpool.tile([128, 1], F32)
    nc.sync.dma_start(bt[:], b_in[:])

    # ---- Phase 1: matvec y[row] = x[row, :] . w + bd ----
    y_all = spool.tile([128, NLANE], F32)     # column i = rows [128i, 128(i+1))
    n_big = ROWS // (128 * TILE_A)            # 32 DMA tiles of 1 MiB
    for i in range(n_big):
        xt = xpool.tile([128, TILE_A * D], F32)
        src = x_in[i * 128 * TILE_A:(i + 1) * 128 * TILE_A, :] \
            .rearrange("(a p) d -> p a d", a=TILE_A)
        nc.sync.dma_start(xt[:].rearrange("p (a d) -> p a d", a=TILE_A), src)
        for a in range(TILE_A):
            dummy = ppool.tile([128, 1], F32)
            col = i * TILE_A + a
            nc.vector.scalar_tensor_tensor(
                out=dummy[:].broadcast_to([128, D]),
                in0=xt[:, a * D:(a + 1) * D], scalar=1.0, in1=wt[:],
                op0=Alu.mult, op1=Alu.mult,
                accum_out=y_all[:, col:col + 1])

    # ---- Phase 2a: transpose y_all -> ys [lane, t-within-chunk] ----
    ident = wpool.tile([128, 128], F32)
    make_identity(nc, ident[:])
    ps = psum.tile([128, 128], F32)
    nc.tensor.transpose(ps[:], y_all[:], ident[:])
    ys = spool.tile([128, CHUNK], F32)
    # PSUM -> SBUF copy with the bias difference folded in (bias per lane is
    # the same scalar, passed as a per-partition vector)
    nc.scalar.activation(ys[:], ps[:], Act.Identity, bias=bt[:], scale=1.0)

    # d bits
    dbits = spool.tile([128, CHUNK], F32)
    nc.vector.tensor_scalar(dbits[:], ys[:], 0.0, None, op0=Alu.is_gt)
    db = dbits[:].rearrange("p (b e) -> p b e", e=4)
    bq = [db[:, :, q:q + 1] for q in range(4)]

    # ---- bootstrap 4-step maps: img[q] = first-one position >= q (else 0) ----
    L0 = spool.tile([128, NBLK * PAD], F32)
    t1 = spool.tile([128, NBLK], F32)
    t2 = spool.tile([128, NBLK], F32)
    i3, i2, i1, i0 = (_ej(L0, NBLK, j) for j in range(4)[::-1])
    nc.vector.tensor_scalar(i3, bq[3], 3.0, None, op0=Alu.mult)
    # i2 = i3 - b2*(i3-2)
    nc.vector.tensor_scalar(t1[:], i3, 2.0, None, op0=Alu.subtract)
    nc.vector.tensor_tensor(t2[:], t1[:], bq[2], op=Alu.mult)
    nc.vector.tensor_tensor(i2, i3, t2[:], op=Alu.subtract)
    # i1 = i2 - b1*(i2-1)
    nc.vector.tensor_scalar(t1[:], i2, 1.0, None, op0=Alu.subtract)
    nc.vector.tensor_tensor(t2[:], t1[:], bq[1], op=Alu.mult)
    nc.vector.tensor_tensor(i1, i2, t2[:], op=Alu.subtract)
    # i0 = i1 - b0*i1
    nc.vector.tensor_tensor(t2[:], i1, bq[0], op=Alu.mult)
    nc.vector.tensor_tensor(i0, i1, t2[:], op=Alu.subtract)

    # ---- in-lane up-sweep: 32 -> 16 -> 8 -> 4 -> 2 -> 1 blocks ----
    cmp_s = spool.tile([128, NBLK * PAD], mybir.dt.uint8)
    ups = [L0]
    n = NBLK
    while n > 1:
        n //= 2
        Ln = spool.tile([128, max(n, 1) * PAD], F32, tag=f"L{n}")
        prev = ups[-1]
        pv = prev[:].rearrange("p (b e) -> p b e", e=PAD)
        # even/odd block views of prev as padded "tiles" via AP slicing
        _combine_views(nc, cmp_s, Ln, pv[:, 1::2, 0:4], pv[:, 0::2, 0:4], n)
        ups.append(Ln)

    # ---- cross-lane scan over 128 lane summaries (64 per sequence) ----
    Sc = spool.tile([2, LANES_PER_SEQ * 4], F32)     # contiguous staging
    L5 = ups[-1]
    for s in range(SEQ_PER_CORE):
        nc.sync.dma_start(Sc[s:s + 1, :], L5[64 * s:64 * (s + 1), 0:4])
    S0 = spool.tile([2, LANES_PER_SEQ * PAD], F32)
    nc.vector.tensor_copy(_ev(S0, LANES_PER_SEQ)[:2],
                          Sc[:].rearrange("p (b e) -> p b e", e=4))
    cmp2 = spool.tile([2, LANES_PER_SEQ * PAD], mybir.dt.uint8)
    sups = [S0]
    n = LANES_PER_SEQ
    while n > 1:
        n //= 2
        Sn = spool.tile([2, max(n, 1) * PAD], F32, tag=f"S{n}")
        pv = sups[-1][:].rearrange("p (b e) -> p b e", e=PAD)
        _combine_views(nc, cmp2, Sn, pv[:2, 1::2, 0:4], pv[:2, 0::2, 0:4], n, p=2)
        sups.append(Sn)
    # down-sweep: exclusive prefixes; root = identity map [0,1,2,3]
    En = spool.tile([2, PAD], F32)
    idt = spool.tile([2, 4], mybir.dt.int32)
    nc.gpsimd.iota(idt[:], pattern=[[1, 4]], channel_multiplier=0)
    nc.vector.tensor_copy(En[:, 0:4], idt[:])
    for lvl in range(len(sups) - 2, -1, -1):
        n = LANES_PER_SEQ >> lvl
        Ed = spool.tile([2, n * PAD], F32, tag=f"SE{n}")
        ev = Ed[:].rearrange("p (b e) -> p b e", e=PAD)
        pv = sups[lvl][:].rearrange("p (b e) -> p b e", e=PAD)
        par = En[:].rearrange("p (b e) -> p b e", e=PAD)
        nc.vector.tensor_copy(ev[:2, 0::2, 0:4], par[:2, :, 0:4])
        _combine_views(nc, cmp2, None, pv[:2, 0::2, 0:4], par[:2, :, 0:4],
                       n // 2, p=2, out_view=ev[:2, 1::2, 0:4])
        En = Ed
    # En now [2, 64*PAD] exclusive lane-entry maps; ship back to lanes
    Ec = spool.tile([2, LANES_PER_SEQ * 4], F32)
    nc.vector.tensor_copy(Ec[:].rearrange("p (b e) -> p b e", e=4),
                          En[:].rearrange("p (b e) -> p b e", e=PAD)[:2, :, 0:4])
    Elane = spool.tile([128, PAD], F32)
    for s in range(SEQ_PER_CORE):
        nc.sync.dma_start(Elane[64 * s:64 * (s + 1), 0:4], Ec[s:s + 1, :])

    # ---- in-lane down-sweep: 1 -> 2 -> ... -> 32 blocks ----
    Ecur = Elane
    for lvl in range(len(ups) - 2, -1, -1):
        n = NBLK >> lvl
        Ed = spool.tile([128, n * PAD], F32, tag=f"E{n}")
        ev = Ed[:].rearrange("p (b e) -> p b e", e=PAD)
        pv = ups[lvl][:].rearrange("p (b e) -> p b e", e=PAD)
        par = Ecur[:].rearrange("p (b e) -> p b e", e=PAD)
        nc.vector.tensor_copy(ev[:, 0::2, 0:4], par[:, :, 0:4])
        _combine_views(nc, cmp_s, None, pv[:, 0::2, 0:4], par[:, :, 0:4],
                       n // 2, out_view=ev[:, 1::2, 0:4])
        Ecur = Ed

    # ---- per-position masks from block-entry state s and bits ----
    sblk = _ej(Ecur, NBLK, 0)                 # flag at each 4-block start
    mask = spool.tile([128, CHUNK], F32)
    mv = mask[:].rearrange("p (b e) -> p b e", e=4)
    mq = [mv[:, :, q:q + 1] for q in range(4)]
    c1 = spool.tile([128, NBLK], F32)
    c2 = spool.tile([128, NBLK], F32)
    c12 = spool.tile([128, NBLK], F32)
    e0 = spool.tile([128, NBLK], F32)
    e1 = spool.tile([128, NBLK], F32)
    e2 = spool.tile([128, NBLK], F32)
    u = spool.tile([128, NBLK], F32)
    nc.vector.tensor_tensor(c1[:], bq[0], bq[1], op=Alu.max)
    nc.vector.tensor_tensor(c2[:], c1[:], bq[2], op=Alu.max)
    nc.vector.tensor_tensor(c12[:], bq[1], bq[2], op=Alu.max)
    nc.vector.tensor_scalar(e0[:], sblk, 0.0, None, op0=Alu.is_equal)
    nc.vector.tensor_scalar(e1[:], sblk, 1.0, None, op0=Alu.is_equal)
    nc.vector.tensor_scalar(e2[:], sblk, 2.0, None, op0=Alu.is_equal)
    # m0 = s > 0
    nc.vector.tensor_scalar(mq[0], sblk, 0.0, None, op0=Alu.is_gt)
    # m1 = max(s > 1, e0*b0)
    nc.vector.tensor_tensor(u[:], e0[:], bq[0], op=Alu.mult)
    nc.vector.tensor_scalar(mq[1], sblk, 1.0, None, op0=Alu.is_gt)
    nc.vector.tensor_tensor(mq[1], mv[:, :, 1:2], u[:], op=Alu.max)
    # m2 = max(s > 2, e0*c1, e1*b1)
    nc.vector.tensor_scalar(mq[2], sblk, 2.0, None, op0=Alu.is_gt)
    nc.vector.tensor_tensor(u[:], e0[:], c1[:], op=Alu.mult)
    nc.vector.tensor_tensor(mq[2], mv[:, :, 2:3], u[:], op=Alu.max)
    nc.vector.tensor_tensor(u[:], e1[:], bq[1], op=Alu.mult)
    nc.vector.tensor_tensor(mq[2], mv[:, :, 2:3], u[:], op=Alu.max)
    # m3 = max(e0*c2, e1*c12, e2*b2)
    nc.vector.tensor_tensor(u[:], e0[:], c2[:], op=Alu.mult)
    nc.vector.tensor_copy(mq[3], u[:])
    nc.vector.tensor_tensor(u[:], e1[:], c12[:], op=Alu.mult)
    nc.vector.tensor_tensor(mq[3], mv[:, :, 3:4], u[:], op=Alu.max)
    nc.vector.tensor_tensor(u[:], e2[:], bq[2], op=Alu.mult)
    nc.vector.tensor_tensor(mq[3], mv[:, :, 3:4], u[:], op=Alu.max)

    # ---- epilogue ----
    dm = spool.tile([128, CHUNK], F32)
    nc.vector.scalar_tensor_tensor(
        out=dm[:], in0=mask[:], scalar=NEG, in1=ys[:],
        op0=Alu.mult, op1=Alu.add)
    bnd = spool.tile([128, CHUNK], F32)
    nc.vector.tensor_scalar(bnd[:], dm[:], 0.0, None, op0=Alu.is_gt)
    ab = spool.tile([128, CHUNK], F32)
    nc.scalar.activation(ab[:], dm[:], Act.Abs)
    ex = spool.tile([128, CHUNK], F32)
    nc.scalar.activation(ex[:], ab[:], Act.Exp, scale=-1.0 / TAU)
    lg = spool.tile([128, CHUNK], F32)
    nc.scalar.activation(lg[:], ex[:], Act.Ln, bias=1.0)
    gth = spool.tile([128, CHUNK], F32)
    nc.scalar.activation(gth[:], lg[:], Act.Copy, scale=-1.0)

    bdst = bnd_out.rearrange("s (c w) -> (s c) w", w=CHUNK)
    gdst = gth_out.rearrange("s (c w) -> (s c) w", w=CHUNK)
    nc.sync.dma_start(bdst, bnd[:])
    nc.sync.dma_start(gdst, gth[:])


def _combine_views(nc, scratch, Cout, Aview, Bview, n, p=128, out_view=None):
    """C = A o B with explicit [p, n, 4] views (padded layouts)."""
    cC = out_view if out_view is not None else _ev(Cout, n)
    if out_view is None and p != 128:
        cC = cC[:p]
    if n == 1:
        # flat 2-D path: [p, 4] operands, entry-j data broadcast via stride-0
        cC = cC[:, 0, :]
        Bv = Bview[:, 0, :]
        cmp = scratch[:].rearrange("p (b e) -> p b e", e=PAD)[:p, 0, 0:4]
        nc.vector.tensor_copy(cC, Aview[:, 0, 0:1].broadcast_to([p, 4]))
        for j in (1, 2, 3):
            nc.vector.tensor_scalar(cmp, Bv, float(j), None, op0=Alu.is_equal)
            nc.vector.copy_predicated(
                cC, cmp, Aview[:, 0, j:j + 1].broadcast_to([p, 4]))
        return
    cmp = scratch[:].rearrange("p (b e) -> p b e", e=PAD)[:p, :n, 0:4]
    a0 = Aview[:, :, 0:1]
    nc.vector.tensor_copy(cC, a0.broadcast_to([p, n, 4]))
    for j in (1, 2, 3):
        nc.vector.tensor_scalar(cmp, Bview, float(j), None, op0=Alu.is_equal)
        aj = Aview[:, :, j:j + 1]
        nc.vector.copy_predicated(cC, cmp, aj.broadcast_to([p, n, 4]))


def build_program():
    nc = bacc.Bacc()
    x_in = nc.declare_dram_parameter("x", [ROWS, D], F32, isOutput=False)
    w_in = nc.declare_dram_parameter("w", [128, D], F32, isOutput=False)
    b_in = nc.declare_dram_parameter("bias", [128, 1], F32, isOutput=False)
    bnd_out = nc.declare_dram_parameter("bnd", [SEQ_PER_CORE, L], F32, isOutput=True)
    gth_out = nc.declare_dram_parameter("gth", [SEQ_PER_CORE, L], F32, isOutput=True)
    with tile.TileContext(nc) as tc:
        _program(tc, x_in[:], w_in[:], b_in[:], bnd_out[:], gth_out[:])
    nc.compile()
    return nc


_NC_CACHE = None


def kernel(x, label, W, b, _trace=False, _tmpdir=None):
    global _NC_CACHE
    x = np.ascontiguousarray(np.asarray(x, dtype=np.float32))
    W = np.asarray(W, dtype=np.float32)
    b = np.asarray(b, dtype=np.float32)
    wd = np.ascontiguousarray(np.repeat((W[:, 1] - W[:, 0])[None, :], 128, axis=0))
    bd = np.full((128, 1), np.float32(b[1] - b[0]), dtype=np.float32)

    if _NC_CACHE is None:
        _NC_CACHE = build_program()
    nc = _NC_CACHE

    in_maps = []
    for c in range(N_CORES):
        shard = x[c * SEQ_PER_CORE:(c + 1) * SEQ_PER_CORE].reshape(ROWS, D)
        in_maps.append({"x": np.ascontiguousarray(shard), "w": wd, "bias": bd})

    res = run_bass_kernel_spmd(nc, in_maps, list(range(N_CORES)),
                               trace=_trace, tmpdir=_tmpdir)
    boundaries = np.concatenate(
        [res.results[c]["bnd"] for c in range(N_CORES)], axis=0)
    gathered = np.concatenate(
        [res.results[c]["gth"] for c in range(N_CORES)], axis=0)[..., None]
    out = (boundaries.astype(np.float32), gathered.astype(np.float32))
    if _trace:
        return out, res
    return out


# revision 22
# speedup vs baseline: 1.0263x; 1.0263x over previous
"""Trainium2 Bass kernel for nn_BiDiBoundaryPredictor.

Math: logits = x @ W + b; a sequential per-timestep scan adds NEG=-10000 to the
boundary logit while a 4-state refractory counter (flag in {0..3}) is nonzero;
outputs are argmax over the masked logits and the log-softmax probability at
the argmax (temperature TAU=2).

Only the logit DIFFERENCE delta = x . (W[:,1]-W[:,0]) + (b1-b0) matters:
  pred_t      = (delta_t + NEG*mask_t) > 0
  gathered_t  = -ln(1 + exp(-|delta_t + NEG*mask_t| / TAU))
  mask_t      = (flag_t > 0), flag driven by bits d_t = delta_t > 0.

Per-core plan (8 cores, 2 sequences each; data-parallel over batch):
  Stream (DMA-bound ~103us): x in 1 MiB tiles; DVE scalar_tensor_tensor with
    accum computes y = sum(x*w) per 128-timestep column -> y_all [128,128].
    Slab PE transposes (y_all -> ys[lane, t]) overlap the stream.
  Tail: the 4-state automaton as an associative map composition, via
    Hillis-Steele scans over map images (4 floats per map): bootstrap 4-step
    blocks with a first-one formula, in-lane HS over 32 blocks, cross-lane HS
    over 64 chunk summaries per sequence, one apply step, closed-form
    per-position masks, then Abs/Exp/Ln epilogue on ACT.
"""
import numpy as np
from contextlib import ExitStack

import concourse.bass as bass
import concourse.tile as tile
from concourse import bacc, mybir
from concourse._compat import with_exitstack
from concourse.masks import make_identity
from concourse.bass_utils import run_bass_kernel_spmd

F32 = mybir.dt.float32
U8 = mybir.dt.uint8
Alu = mybir.AluOpType
Act = mybir.ActivationFunctionType

NEG = -10000.0
TAU = 2.0
N_CORES = 8
BS, L, D = 16, 8192, 512
SEQ_PER_CORE = BS // N_CORES          # 2
ROWS = SEQ_PER_CORE * L               # 16384 x-rows per core
CHUNK = 128                           # timesteps per lane
NLANE = ROWS // CHUNK                 # 128 lanes = (seq, chunk)
LANES_PER_SEQ = L // CHUNK            # 64
TILE_A = 4                            # 128-row chunks per DMA tile (1 MiB)
NBLK = CHUNK // 4                     # 32 four-step blocks per lane
PAD = 5                               # padded block stride for map storage


def _pv(t, n):
    """[P, n, 4] entry view of a PAD-strided map tile (or a slice of it)."""
    return t[:].rearrange("p (b e) -> p b e", e=PAD)[:, :n, 0:4]


def _combine(nc, cmp, Cv, Av, Bv, p, n):
    """Map composition C = A o B (C[blk, s] = A[blk, B[blk, s]]).

    A is the later map, B the earlier; all views [p, n, 4]; cmp is a uint8
    padded scratch tile. 7 DVE ops (compare + predicated copy).
    """
    cv = cmp[:].rearrange("p (b e) -> p b e", e=PAD)[:p, :n, 0:4]
    nc.vector.tensor_copy(Cv, Av[:, :, 0:1].broadcast_to([p, n, 4]))
    for j in (1, 2, 3):
        nc.vector.tensor_scalar(cv, Bv, float(j), None, op0=Alu.is_equal)
        nc.vector.copy_predicated(Cv, cv, Av[:, :, j:j + 1].broadcast_to([p, n, 4]))


@with_exitstack
def _program(ctx: ExitStack, tc: tile.TileContext,
             x_in, w_in, b_in, bnd_out, gth_out):
    nc = tc.nc

    xpool = ctx.enter_context(tc.tile_pool(name="x", bufs=4))
    ppool = ctx.enter_context(tc.tile_pool(name="prod", bufs=2))
    wpool = ctx.enter_context(tc.tile_pool(name="w", bufs=1))
    spool = ctx.enter_context(tc.tile_pool(name="scan", bufs=1))
    psum = ctx.enter_context(tc.tile_pool(name="ps", bufs=2, space="PSUM"))

    wt = wpool.tile([128, D], F32)
    nc.sync.dma_start(wt[:], w_in[:])
    bt = wpool.tile([128, 1], F32)
    nc.sync.dma_start(bt[:], b_in[:])

    # constants + one-time warmups (overlap the stream)
    ident = wpool.tile([128, 128], F32)
    make_identity(nc, ident[:])
    idt = spool.tile([2, 4], mybir.dt.int32)
    nc.gpsimd.iota(idt[:], pattern=[[1, 4]], channel_multiplier=0)
    idf = spool.tile([2, 4], F32)
    nc.vector.tensor_copy(idf[:], idt[:])
    warm = spool.tile([128, 1], F32)
    nc.scalar.activation(warm[:], bt[:], Act.Exp, scale=0.0)  # ACT table preload

    # ---- stream: matvec y[row] = x[row, :] . w  (DVE, under DMA shadow) ----
    y_all = spool.tile([128, NLANE], F32)     # column i = rows [128i, 128(i+1))
    ys = spool.tile([128, CHUNK], F32)        # lane-major, after transpose
    dbits = spool.tile([128, CHUNK], F32)
    n_big = ROWS // (128 * TILE_A)            # 32 DMA tiles of 1 MiB
    slab_end = {7: 0, 15: 1, 23: 2, 31: 3}
    for i in range(n_big):
        xt = xpool.tile([128, TILE_A * D], F32)
        src = x_in[i * 128 * TILE_A:(i + 1) * 128 * TILE_A, :] \
            .rearrange("(a p) d -> p a d", a=TILE_A)
        nc.sync.dma_start(xt[:].rearrange("p (a d) -> p a d", a=TILE_A), src)
        for a in range(TILE_A):
            dummy = ppool.tile([128, 1], F32)
            col = i * TILE_A + a
            nc.vector.scalar_tensor_tensor(
                out=dummy[:].broadcast_to([128, D]),
                in0=xt[:, a * D:(a + 1) * D], scalar=1.0, in1=wt[:],
                op0=Alu.mult, op1=Alu.mult,
                accum_out=y_all[:, col:col + 1])
        s = slab_end.get(i)
        if s is not None:
            # 32 lanes finished: fold bias in, transpose to PSUM partition 0
            # (walrus requires it), stage to SBUF, DMA-scatter to lane rows
            sl = slice(32 * s, 32 * (s + 1))
            nc.vector.tensor_scalar(y_all[:, sl], y_all[:, sl], bt[:],
                                    None, op0=Alu.add)
            ps_t = psum.tile([32, 128], F32, tag="pslab")
            nc.tensor.transpose(ps_t[:], y_all[:, sl], ident[:])
            stg = xpool.tile([32, 128], F32, tag="stg", bufs=2)
            nc.scalar.activation(stg[:], ps_t[:], Act.Identity, scale=1.0)
            nc.sync.dma_start(ys[sl, :], stg[:])
            nc.gpsimd.tensor_scalar(dbits[sl, :], ys[sl, :], 0.0, None,
                                    op0=Alu.is_gt)

    # ---- bootstrap 4-step maps: img[q] = first-one position >= q (else 0) ----
    db = dbits[:].rearrange("p (b e) -> p b e", e=4)
    bq = [db[:, :, q:q + 1] for q in range(4)]
    H0 = spool.tile([128, NBLK * PAD], F32)
    H1 = spool.tile([128, NBLK * PAD], F32)
    t1 = spool.tile([128, NBLK], F32)
    t2 = spool.tile([128, NBLK], F32)
    hv = H0[:].rearrange("p (b e) -> p b e", e=PAD)
    i3, i2, i1, i0 = (hv[:, :, j:j + 1] for j in (3, 2, 1, 0))
    nc.vector.tensor_scalar(i3, bq[3], 3.0, None, op0=Alu.mult)
    nc.vector.tensor_scalar(t1[:], i3, 2.0, None, op0=Alu.subtract)
    nc.vector.tensor_tensor(t2[:], t1[:], bq[2], op=Alu.mult)
    nc.vector.tensor_tensor(i2, i3, t2[:], op=Alu.subtract)
    nc.vector.tensor_scalar(t1[:], i2, 1.0, None, op0=Alu.subtract)
    nc.vector.tensor_tensor(t2[:], t1[:], bq[1], op=Alu.mult)
    nc.vector.tensor_tensor(i1, i2, t2[:], op=Alu.subtract)
    nc.vector.tensor_tensor(t2[:], i1, bq[0], op=Alu.mult)
    nc.vector.tensor_tensor(i0, i1, t2[:], op=Alu.subtract)

    # ---- in-lane Hillis-Steele over 32 blocks (inclusive prefixes) ----
    cmp_s = spool.tile([128, NBLK * PAD], U8)
    cur, nxt = H0, H1
    for k in (1, 2, 4, 8, 16):
        cv = _pv(nxt, NBLK)[:, k:, :]
        av = _pv(cur, NBLK)[:, k:, :]
        bv = _pv(cur, NBLK)[:, :NBLK - k, :]
        _combine(nc, cmp_s, cv, av, bv, 128, NBLK - k)
        nc.vector.tensor_copy(_pv(nxt, NBLK)[:, :k, :], _pv(cur, NBLK)[:, :k, :])
        cur, nxt = nxt, cur
    Hf = cur                                   # inclusive in-lane prefixes

    # ---- cross-lane HS over the 64 chunk summaries of each sequence ----
    Sc = spool.tile([2, LANES_PER_SEQ * 4], F32)
    for s in range(SEQ_PER_CORE):
        nc.sync.dma_start(
            Sc[s:s + 1, :],
            Hf[64 * s:64 * (s + 1), 31 * PAD:31 * PAD + 4])
    T0 = spool.tile([2, LANES_PER_SEQ * PAD], F32)
    T1 = spool.tile([2, LANES_PER_SEQ * PAD], F32)
    cmp2 = spool.tile([2, LANES_PER_SEQ * PAD], U8)
    nc.vector.tensor_copy(_pv(T0, LANES_PER_SEQ)[:2],
                          Sc[:].rearrange("p (b e) -> p b e", e=4))
    M = LANES_PER_SEQ
    cur, nxt = T0, T1
    for k in (1, 2, 4, 8, 16, 32):
        cv = _pv(nxt, M)[:2, k:, :]
        av = _pv(cur, M)[:2, k:, :]
        bv = _pv(cur, M)[:2, :M - k, :]
        _combine(nc, cmp2, cv, av, bv, 2, M - k)
        nc.vector.tensor_copy(_pv(nxt, M)[:2, :k, :], _pv(cur, M)[:2, :k, :])
        cur, nxt = nxt, cur
    Sf = cur                                   # inclusive chunk prefixes

    # exclusive lane-entry maps: lane l gets inclusive[l-1]; lane 0 identity
    Elane = spool.tile([128, PAD], F32)
    for s in range(SEQ_PER_CORE):
        nc.sync.dma_start(Elane[64 * s:64 * s + 1, 0:4], idf[s:s + 1, :])
        nc.sync.dma_start(
            Elane[64 * s + 1:64 * (s + 1), 0:4],
            _pv(Sf, M)[s:s + 1, 0:M - 1, :])

    # ---- apply: global exclusive prefix per 4-block ----
    E0 = spool.tile([128, NBLK * PAD], F32)
    ev = E0[:].rearrange("p (b e) -> p b e", e=PAD)
    el = Elane[:].rearrange("p (b e) -> p b e", e=PAD)[:, 0:1, 0:4]
    nc.vector.tensor_copy(ev[:, 0:1, 0:4], el)
    _combine(nc, cmp_s, ev[:, 1:NBLK, 0:4], _pv(Hf, NBLK)[:, :NBLK - 1, :],
             el.broadcast_to([128, NBLK - 1, 4]), 128, NBLK - 1)

    # ---- per-position masks from block-entry state s and bits ----
    sblk = ev[:, :, 0:1]                      # flag at each 4-block start
    mask = spool.tile([128, CHUNK], F32)
    mv = mask[:].rearrange("p (b e) -> p b e", e=4)
    mq = [mv[:, :, q:q + 1] for q in range(4)]
    c1 = spool.tile([128, NBLK], F32)
    c2 = spool.tile([128, NBLK], F32)
    c12 = spool.tile([128, NBLK], F32)
    e0 = spool.tile([128, NBLK], F32)
    e1 = spool.tile([128, NBLK], F32)
    e2 = spool.tile([128, NBLK], F32)
    u = spool.tile([128, NBLK], F32)
    nc.vector.tensor_tensor(c1[:], bq[0], bq[1], op=Alu.max)
    nc.vector.tensor_tensor(c2[:], c1[:], bq[2], op=Alu.max)
    nc.vector.tensor_tensor(c12[:], bq[1], bq[2], op=Alu.max)
    nc.vector.tensor_scalar(e0[:], sblk, 0.0, None, op0=Alu.is_equal)
    nc.vector.tensor_scalar(e1[:], sblk, 1.0, None, op0=Alu.is_equal)
    nc.vector.tensor_scalar(e2[:], sblk, 2.0, None, op0=Alu.is_equal)
    nc.vector.tensor_scalar(mq[0], sblk, 0.0, None, op0=Alu.is_gt)
    nc.vector.tensor_tensor(u[:], e0[:], bq[0], op=Alu.mult)
    nc.vector.tensor_scalar(mq[1], sblk, 1.0, None, op0=Alu.is_gt)
    nc.vector.tensor_tensor(mq[1], mv[:, :, 1:2], u[:], op=Alu.max)
    nc.vector.tensor_scalar(mq[2], sblk, 2.0, None, op0=Alu.is_gt)
    nc.vector.tensor_tensor(u[:], e0[:], c1[:], op=Alu.mult)
    nc.vector.tensor_tensor(mq[2], mv[:, :, 2:3], u[:], op=Alu.max)
    nc.vector.tensor_tensor(u[:], e1[:], bq[1], op=Alu.mult)
    nc.vector.tensor_tensor(mq[2], mv[:, :, 2:3], u[:], op=Alu.max)
    nc.vector.tensor_tensor(u[:], e0[:], c2[:], op=Alu.mult)
    nc.vector.tensor_copy(mq[3], u[:])
    nc.vector.tensor_tensor(u[:], e1[:], c12[:], op=Alu.mult)
    nc.vector.tensor_tensor(mq[3], mv[:, :, 3:4], u[:], op=Alu.max)
    nc.vector.tensor_tensor(u[:], e2[:], bq[2], op=Alu.mult)
    nc.vector.tensor_tensor(mq[3], mv[:, :, 3:4], u[:], op=Alu.max)

    # ---- epilogue ----
    dm = spool.tile([128, CHUNK], F32)
    nc.vector.scalar_tensor_tensor(
        out=dm[:], in0=mask[:], scalar=NEG, in1=ys[:],
        op0=Alu.mult, op1=Alu.add)
    bnd = spool.tile([128, CHUNK], F32)
    nc.vector.tensor_scalar(bnd[:], dm[:], 0.0, None, op0=Alu.is_gt)
    ab = spool.tile([128, CHUNK], F32)
    nc.scalar.activation(ab[:], dm[:], Act.Abs)
    ex = spool.tile([128, CHUNK], F32)
    nc.scalar.activation(ex[:], ab[:], Act.Exp, scale=-1.0 / TAU)
    lg = spool.tile([128, CHUNK], F32)
    nc.scalar.activation(lg[:], ex[:], Act.Ln, bias=1.0)
    gth = spool.tile([128, CHUNK], F32)
    nc.scalar.activation(gth[:], lg[:], Act.Copy, scale=-1.0)

    bdst = bnd_out.rearrange("s (c w) -> (s c) w", w=CHUNK)
    gdst = gth_out.rearrange("s (c w) -> (s c) w", w=CHUNK)
    nc.sync.dma_start(bdst, bnd[:])
    nc.sync.dma_start(gdst, gth[:])


def build_program():
    nc = bacc.Bacc()
    x_in = nc.declare_dram_parameter("x", [ROWS, D], F32, isOutput=False)
    w_in = nc.declare_dram_parameter("w", [128, D], F32, isOutput=False)
    b_in = nc.declare_dram_parameter("bias", [128, 1], F32, isOutput=False)
    bnd_out = nc.declare_dram_parameter("bnd", [SEQ_PER_CORE, L], F32, isOutput=True)
    gth_out = nc.declare_dram_parameter("gth", [SEQ_PER_CORE, L], F32, isOutput=True)
    with tile.TileContext(nc) as tc:
        _program(tc, x_in[:], w_in[:], b_in[:], bnd_out[:], gth_out[:])
    nc.compile()
    return nc


_NC_CACHE = None


def kernel(x, label, W, b, _trace=False, _tmpdir=None):
    global _NC_CACHE
    x = np.ascontiguousarray(np.asarray(x, dtype=np.float32))
    W = np.asarray(W, dtype=np.float32)
    b = np.asarray(b, dtype=np.float32)
    wd = np.ascontiguousarray(np.repeat((W[:, 1] - W[:, 0])[None, :], 128, axis=0))
    bd = np.full((128, 1), np.float32(b[1] - b[0]), dtype=np.float32)

    if _NC_CACHE is None:
        _NC_CACHE = build_program()
    nc = _NC_CACHE

    in_maps = []
    for c in range(N_CORES):
        shard = x[c * SEQ_PER_CORE:(c + 1) * SEQ_PER_CORE].reshape(ROWS, D)
        in_maps.append({"x": np.ascontiguousarray(shard), "w": wd, "bias": bd})

    res = run_bass_kernel_spmd(nc, in_maps, list(range(N_CORES)),
                               trace=_trace, tmpdir=_tmpdir)
    boundaries = np.concatenate(
        [res.results[c]["bnd"] for c in range(N_CORES)], axis=0)
    gathered = np.concatenate(
        [res.results[c]["gth"] for c in range(N_CORES)], axis=0)[..., None]
    out = (boundaries.astype(np.float32), gathered.astype(np.float32))
    if _trace:
        return out, res
    return out
